# revision 3
# baseline (speedup 1.0000x reference)
"""Trainium2 Bass kernel for nn_AttributeBranch.

Math: the CBP (count-sketch -> rFFT -> elementwise mul -> irFFT) block is,
per sample, a LINEAR operator on the channel dim:
    mcb[b] = Circ(sk1[b]) @ P2^T @ feature[b]
where Circ(v)[n, e] = v[(n-e) mod D] (circular convolution) and sk1[b]
depends only on attr_one_hot[b].  Folding conv1_w in gives a per-sample
[32, 256] matrix G_b:
    x[b]       = relu(G_b @ relu(E_b) + c1)          # E_b = entity[b] as [256, HW]
    attr_map   = sigmoid(conv2_w @ x + c2)           # [1, HW]
    attr_feat  = attr_map * E_b
G_b = L1 @ (Fr P2^T) + L2 @ (Fi P2^T), with
    L1 = u * f1re[b] + v * f1im[b],  L2 = v * f1re[b] - u * f1im[b]
    u = conv1_w @ Br, v = conv1_w @ Bi   (full 256-bin real DFT matrices)
and the whole attr_one_hot -> f1re/f1im chain collapses into two
parameter-only [256, 512] matrices (augmented with a bias row).

All data-dependent compute runs on the NeuronCores.  Host only folds
*parameter* matrices and re-lays-out tensors.  Sharding: pure data parallel
over batch (4 samples per core x 8 cores).
"""

import os

import numpy as np

D = 256
H = 56
W = 56
HW = H * W
B = 32
ATTR = 400
NCORES = 8
BL = B // NCORES          # samples per core
NT = 7                    # spatial tiles per sample
TW = HW // NT             # 448 columns per tile
KAUG = 512                # padded (attr_num + bias row) contraction dim

LAST_RESULT = None        # BassKernelResults of the most recent kernel() call


def _swizzle_k(mat):
    """[K, M] with K = n*128 -> [128, n, M] so that dev[pi, po, m] = mat[po*128+pi, m]."""
    k, m = mat.shape
    n = k // 128
    return np.ascontiguousarray(
        mat.reshape(n, 128, m).transpose(1, 0, 2)
    ).astype(np.float32)


def _host_constants(W_emb, b_emb, conv_w, conv_b, conv1_w, conv1_b, conv2_w,
                    conv2_b, h1, h2, s1, s2):
    """Fold all parameter-only matrices (float64 for accuracy)."""
    f8 = np.float64

    def sketch(h, s):
        P = np.zeros((D, D), f8)
        np.add.at(P, (np.arange(D), np.asarray(h)), np.asarray(s, f8))
        return P

    P1m = sketch(h1, s1)
    P2m = sketch(h2, s2)

    k = np.arange(D)
    ang = 2.0 * np.pi * np.outer(k, k) / D
    Fr = np.cos(ang)
    Fi = -np.sin(ang)
    Br = Fr / D
    Bi = Fi / D

    conv_w = np.asarray(conv_w, f8)
    W_emb = np.asarray(W_emb, f8)
    conv1_w = np.asarray(conv1_w, f8)

    u = conv1_w @ Br                      # [32, 256]
    v = conv1_w @ Bi
    FrP = Fr @ P2m.T                      # [256, 256]
    FiP = Fi @ P2m.T

    # attr_one_hot -> f1{re,im} collapsed:  f1 = aoh_aug @ M_aug.T
    pre = P1m.T @ (conv_w @ W_emb)        # [256, 400]
    cvec = P1m.T @ (conv_w @ np.asarray(b_emb, f8) + np.asarray(conv_b, f8))
    Mre = np.zeros((D, KAUG), f8)
    Mim = np.zeros((D, KAUG), f8)
    Mre[:, :ATTR] = Fr @ pre
    Mre[:, ATTR] = Fr @ cvec
    Mim[:, :ATTR] = Fi @ pre
    Mim[:, ATTR] = Fi @ cvec

    return {
        "mre": _swizzle_k(Mre.T),                      # [128, 4, 256]
        "mim": _swizzle_k(Mim.T),
        "frp": _swizzle_k(FrP),                        # [128, 2, 256]
        "fip": _swizzle_k(FiP),
        "ut": _swizzle_k(u.T),                         # [128, 2, 32]
        "vt": _swizzle_k(v.T),
        "w2b": np.tile(
            np.asarray(conv2_w, np.float64).reshape(32, 1), (1, 128)
        ).astype(np.float32),                          # [32, 128]
        "c1b": np.asarray(conv1_b, np.float32).reshape(32, 1),
        "c2b": np.full((128, 1), float(np.asarray(conv2_b).reshape(-1)[0]),
                       np.float32),
    }


def _build_nc(mm_f32r=True):
    import concourse.bacc as bacc
    import concourse.mybir as mybir
    import concourse.tile as tile

    F32 = mybir.dt.float32
    MMDT = mybir.dt.float32r if mm_f32r else mybir.dt.float32
    Alu = mybir.AluOpType
    Act = mybir.ActivationFunctionType

    nc = bacc.Bacc("TRN2", target_bir_lowering=False, debug=False,
                   num_devices=NCORES)

    ef = nc.dram_tensor("ef", [BL, D, HW], F32, kind="ExternalInput")
    aoht = nc.dram_tensor("aoht", [128, 4, BL], F32, kind="ExternalInput")
    d_mre = nc.dram_tensor("mre", [128, 4, D], F32, kind="ExternalInput")
    d_mim = nc.dram_tensor("mim", [128, 4, D], F32, kind="ExternalInput")
    d_frp = nc.dram_tensor("frp", [128, 2, D], F32, kind="ExternalInput")
    d_fip = nc.dram_tensor("fip", [128, 2, D], F32, kind="ExternalInput")
    d_ut = nc.dram_tensor("ut", [128, 2, 32], F32, kind="ExternalInput")
    d_vt = nc.dram_tensor("vt", [128, 2, 32], F32, kind="ExternalInput")
    d_w2b = nc.dram_tensor("w2b", [32, 128], MMDT, kind="ExternalInput")
    d_c1b = nc.dram_tensor("c1b", [32, 1], F32, kind="ExternalInput")
    d_c2b = nc.dram_tensor("c2b", [128, 1], F32, kind="ExternalInput")
    amap = nc.dram_tensor("amap", [BL, 1, HW], F32, kind="ExternalOutput")
    afeat = nc.dram_tensor("afeat", [BL, D, HW], F32, kind="ExternalOutput")

    def r128(ap):
        return ap.rearrange("(po pi) f -> pi po f", pi=128)

    with tile.TileContext(nc) as tc:
        with (
            tc.tile_pool(name="consts", bufs=1) as consts,
            tc.tile_pool(name="work", bufs=3) as work,
            tc.tile_pool(name="big", bufs=2) as big,
            tc.tile_pool(name="outp", bufs=2) as outp,
            tc.tile_pool(name="small", bufs=2) as small,
            tc.tile_pool(name="psA", bufs=2, space="PSUM") as psA,
            tc.tile_pool(name="ps1p", bufs=2, space="PSUM") as ps1p,
            tc.tile_pool(name="ps2p", bufs=2, space="PSUM") as ps2p,
        ):
            # ---- constant loads (SP HWDGE queue) ----
            def cload(dram, shape, dt=F32):
                t = consts.tile(shape, dt, tag=dram.name)
                nc.sync.dma_start(t[:], dram.ap())
                return t

            aoht_sb = cload(aoht, [128, 4, BL])
            mre_sb = cload(d_mre, [128, 4, D])
            mim_sb = cload(d_mim, [128, 4, D])
            frp_sb = cload(d_frp, [128, 2, D])
            fip_sb = cload(d_fip, [128, 2, D])
            ut_sb = cload(d_ut, [128, 2, 32])
            vt_sb = cload(d_vt, [128, 2, 32])
            w2b_sb = cload(d_w2b, [32, 128], dt=MMDT)
            c1b_sb = cload(d_c1b, [32, 1])
            c2b_sb = cload(d_c2b, [128, 1])

            # ---- stage A: per-sample G matrices (tiny, fp32 exact) ----
            f1re_sb = consts.tile([128, 2, BL], F32, tag="f1re")
            f1im_sb = consts.tile([128, 2, BL], F32, tag="f1im")
            for dst, mat in ((f1re_sb, mre_sb), (f1im_sb, mim_sb)):
                for m in range(2):
                    ps = psA.tile([128, BL], F32, tag="psA")
                    for po in range(4):
                        nc.tensor.matmul(
                            ps[:], lhsT=mat[:, po, m * 128:(m + 1) * 128],
                            rhs=aoht_sb[:, po, :],
                            start=(po == 0), stop=(po == 3))
                    nc.vector.tensor_copy(dst[:, m, :], ps[:])

            GT_sb = consts.tile([128, 2, BL, 32], MMDT, tag="GT")
            for b in range(BL):
                f1re_b = f1re_sb[:, :, b:b + 1].to_broadcast([128, 2, 32])
                f1im_b = f1im_sb[:, :, b:b + 1].to_broadcast([128, 2, 32])
                L1 = work.tile([128, 2, 32], F32, tag="L1")
                L2 = work.tile([128, 2, 32], F32, tag="L2")
                t1 = work.tile([128, 2, 32], F32, tag="Ltmp")
                nc.vector.tensor_tensor(L1[:], ut_sb[:], f1re_b, Alu.mult)
                nc.vector.tensor_tensor(t1[:], vt_sb[:], f1im_b, Alu.mult)
                nc.vector.tensor_add(L1[:], L1[:], t1[:])
                t2 = work.tile([128, 2, 32], F32, tag="Ltmp")
                nc.vector.tensor_tensor(L2[:], vt_sb[:], f1re_b, Alu.mult)
                nc.vector.tensor_tensor(t2[:], ut_sb[:], f1im_b, Alu.mult)
                nc.vector.tensor_tensor(L2[:], L2[:], t2[:], Alu.subtract)
                for m in range(2):
                    ps = psA.tile([128, 32], F32, tag="psG")
                    nc.tensor.matmul(ps[:], lhsT=frp_sb[:, 0, m * 128:(m + 1) * 128],
                                     rhs=L1[:, 0, :], start=True, stop=False)
                    nc.tensor.matmul(ps[:], lhsT=frp_sb[:, 1, m * 128:(m + 1) * 128],
                                     rhs=L1[:, 1, :], start=False, stop=False)
                    nc.tensor.matmul(ps[:], lhsT=fip_sb[:, 0, m * 128:(m + 1) * 128],
                                     rhs=L2[:, 0, :], start=False, stop=False)
                    nc.tensor.matmul(ps[:], lhsT=fip_sb[:, 1, m * 128:(m + 1) * 128],
                                     rhs=L2[:, 1, :], start=False, stop=True)
                    nc.vector.tensor_copy(GT_sb[:, m, b, :], ps[:])

            # ---- stage B: the heavy per-pixel pipeline ----
            for b in range(BL):
                E = big.tile([128, 2, HW], F32, tag="E")
                nc.scalar.dma_start(E[:], r128(ef.ap()[b]))  # ACT HWDGE queue
                O = outp.tile([128, 2, HW], F32, tag="O")
                acc = small.tile([1, HW], F32, tag="acc")
                for j in range(NT):
                    js = slice(j * TW, (j + 1) * TW)
                    r = work.tile([128, 2, TW], MMDT, tag="r")
                    nc.vector.tensor_scalar(r[:], E[:, :, js], 0.0, None,
                                            Alu.max)
                    ps1 = ps1p.tile([32, TW], F32, tag="ps1")
                    for po in range(2):
                        nc.tensor.matmul(
                            ps1[:],
                            lhsT=GT_sb[:, po, b, :],
                            rhs=r[:, po, :],
                            start=(po == 0), stop=(po == 1))
                    xt = work.tile([32, TW], MMDT, tag="x")
                    nc.scalar.activation(xt[:], ps1[:], Act.Relu,
                                         bias=c1b_sb[:])
                    ps2 = ps2p.tile([128, TW], F32, tag="ps2")
                    nc.tensor.matmul(ps2[:], lhsT=w2b_sb[:],
                                     rhs=xt[:],
                                     start=True, stop=True)
                    am = work.tile([128, TW], F32, tag="am")
                    nc.scalar.activation(am[:], ps2[:], Act.Sigmoid,
                                         bias=c2b_sb[:])
                    nc.vector.tensor_tensor(
                        O[:, :, js], E[:, :, js],
                        am[:, None, :].to_broadcast([128, 2, TW]), Alu.mult)
                    nc.gpsimd.tensor_copy(acc[:, js], am[0:1, :])
                nc.gpsimd.dma_start(r128(afeat.ap()[b]), O[:])
                nc.gpsimd.dma_start(amap.ap()[b], acc[:])

    nc.compile()
    return nc


def kernel(entity_feature, attr_one_hot, W_emb, b_emb, conv_w, conv_b,
           conv1_w, conv1_b, conv2_w, conv2_b, h1, h2, s1, s2):
    global LAST_RESULT
    from concourse.bass_utils import run_bass_kernel_spmd

    consts = _host_constants(W_emb, b_emb, conv_w, conv_b, conv1_w, conv1_b,
                             conv2_w, conv2_b, h1, h2, s1, s2)

    ef_full = np.ascontiguousarray(
        np.asarray(entity_feature, np.float32).reshape(B, D, HW))
    aoh_full = np.asarray(attr_one_hot, np.float32)

    in_maps = []
    for c in range(NCORES):
        sl = slice(c * BL, (c + 1) * BL)
        aug = np.zeros((KAUG, BL), np.float32)
        aug[:ATTR] = aoh_full[sl].T
        aug[ATTR] = 1.0
        m = dict(consts)
        m["ef"] = np.ascontiguousarray(ef_full[sl])
        m["aoht"] = np.ascontiguousarray(
            aug.reshape(4, 128, BL).transpose(1, 0, 2))
        in_maps.append(m)

    mm_f32r = os.environ.get("KERNEL_MM_DTYPE", "f32r") == "f32r"
    nc = _build_nc(mm_f32r=mm_f32r)

    trace = bool(int(os.environ.get("KERNEL_TRACE", "0")))
    res = run_bass_kernel_spmd(nc, in_maps, core_ids=list(range(NCORES)),
                               trace=trace)
    LAST_RESULT = res

    amap = np.concatenate([res.results[c]["amap"] for c in range(NCORES)], 0)
    afeat = np.concatenate([res.results[c]["afeat"] for c in range(NCORES)], 0)
    return (np.ascontiguousarray(amap.reshape(B, 1, H, W)),
            np.ascontiguousarray(afeat.reshape(B, D, H, W)))


# revision 5
# speedup vs baseline: 1.1099x; 1.1099x over previous
"""Trainium2 Bass kernel for nn_AttributeBranch.

Math: the CBP (count-sketch -> rFFT -> elementwise mul -> irFFT) block is,
per sample, a LINEAR operator on the channel dim:
    mcb[b] = Circ(sk1[b]) @ P2^T @ feature[b]
where Circ(v)[n, e] = v[(n-e) mod D] (circular convolution) and sk1[b]
depends only on attr_one_hot[b].  Folding conv1_w in gives a per-sample
[32, 256] matrix G_b:
    x[b]       = relu(G_b @ relu(E_b) + c1)          # E_b = entity[b] as [256, HW]
    attr_map   = sigmoid(conv2_w @ x + c2)           # [1, HW]
    attr_feat  = attr_map * E_b
G_b = L1 @ (Fr P2^T) + L2 @ (Fi P2^T), with
    L1 = u * f1re[b] + v * f1im[b],  L2 = v * f1re[b] - u * f1im[b]
    u = conv1_w @ Br, v = conv1_w @ Bi   (full 256-bin real DFT matrices)
and the whole attr_one_hot -> f1re/f1im chain collapses into two
parameter-only [256, 512] matrices (augmented with a bias row).

All data-dependent compute runs on the NeuronCores.  Host only folds
*parameter* matrices and re-lays-out tensors.  Sharding: pure data parallel
over batch (4 samples per core x 8 cores).
"""

import os

import numpy as np

D = 256
H = 56
W = 56
HW = H * W
B = 32
ATTR = 400
NCORES = 8
BL = B // NCORES          # samples per core
NT = 7                    # spatial tiles per sample
TW = HW // NT             # 448 columns per tile
KAUG = 512                # padded (attr_num + bias row) contraction dim

LAST_RESULT = None        # BassKernelResults of the most recent kernel() call


def _swizzle_k(mat):
    """[K, M] with K = n*128 -> [128, n, M] so that dev[pi, po, m] = mat[po*128+pi, m]."""
    k, m = mat.shape
    n = k // 128
    return np.ascontiguousarray(
        mat.reshape(n, 128, m).transpose(1, 0, 2)
    ).astype(np.float32)


def _host_constants(W_emb, b_emb, conv_w, conv_b, conv1_w, conv1_b, conv2_w,
                    conv2_b, h1, h2, s1, s2):
    """Fold all parameter-only matrices (float64 for accuracy)."""
    f8 = np.float64

    def sketch(h, s):
        P = np.zeros((D, D), f8)
        np.add.at(P, (np.arange(D), np.asarray(h)), np.asarray(s, f8))
        return P

    P1m = sketch(h1, s1)
    P2m = sketch(h2, s2)

    k = np.arange(D)
    ang = 2.0 * np.pi * np.outer(k, k) / D
    Fr = np.cos(ang)
    Fi = -np.sin(ang)
    Br = Fr / D
    Bi = Fi / D

    conv_w = np.asarray(conv_w, f8)
    W_emb = np.asarray(W_emb, f8)
    conv1_w = np.asarray(conv1_w, f8)

    u = conv1_w @ Br                      # [32, 256]
    v = conv1_w @ Bi
    FrP = Fr @ P2m.T                      # [256, 256]
    FiP = Fi @ P2m.T

    # attr_one_hot -> f1{re,im} collapsed:  f1 = aoh_aug @ M_aug.T
    pre = P1m.T @ (conv_w @ W_emb)        # [256, 400]
    cvec = P1m.T @ (conv_w @ np.asarray(b_emb, f8) + np.asarray(conv_b, f8))
    Mre = np.zeros((D, KAUG), f8)
    Mim = np.zeros((D, KAUG), f8)
    Mre[:, :ATTR] = Fr @ pre
    Mre[:, ATTR] = Fr @ cvec
    Mim[:, :ATTR] = Fi @ pre
    Mim[:, ATTR] = Fi @ cvec

    return {
        "mre": _swizzle_k(Mre.T),                      # [128, 4, 256]
        "mim": _swizzle_k(Mim.T),
        "frp": _swizzle_k(FrP),                        # [128, 2, 256]
        "fip": _swizzle_k(FiP),
        "ut": _swizzle_k(u.T),                         # [128, 2, 32]
        "vt": _swizzle_k(v.T),
        "w2b": np.tile(
            np.asarray(conv2_w, np.float64).reshape(32, 1), (1, 128)
        ).astype(np.float32),                          # [32, 128]
        "c1b": np.asarray(conv1_b, np.float32).reshape(32, 1),
        "c2b": np.full((128, 1), float(np.asarray(conv2_b).reshape(-1)[0]),
                       np.float32),
    }


def _build_nc(mm_dtype="bf16"):
    import concourse.bacc as bacc
    import concourse.mybir as mybir
    import concourse.tile as tile

    F32 = mybir.dt.float32
    MMDT = {"bf16": mybir.dt.bfloat16,
            "f32r": mybir.dt.float32r,
            "f32": mybir.dt.float32}[mm_dtype]
    Alu = mybir.AluOpType
    Act = mybir.ActivationFunctionType

    nc = bacc.Bacc("TRN2", target_bir_lowering=False, debug=False,
                   num_devices=NCORES)

    ef = nc.dram_tensor("ef", [BL, D, HW], F32, kind="ExternalInput")
    aoht = nc.dram_tensor("aoht", [128, 4, BL], F32, kind="ExternalInput")
    d_mre = nc.dram_tensor("mre", [128, 4, D], F32, kind="ExternalInput")
    d_mim = nc.dram_tensor("mim", [128, 4, D], F32, kind="ExternalInput")
    d_frp = nc.dram_tensor("frp", [128, 2, D], F32, kind="ExternalInput")
    d_fip = nc.dram_tensor("fip", [128, 2, D], F32, kind="ExternalInput")
    d_ut = nc.dram_tensor("ut", [128, 2, 32], F32, kind="ExternalInput")
    d_vt = nc.dram_tensor("vt", [128, 2, 32], F32, kind="ExternalInput")
    d_w2b = nc.dram_tensor("w2b", [32, 128], MMDT, kind="ExternalInput")
    d_c1b = nc.dram_tensor("c1b", [32, 1], F32, kind="ExternalInput")
    d_c2b = nc.dram_tensor("c2b", [128, 1], F32, kind="ExternalInput")
    amap = nc.dram_tensor("amap", [BL, 1, HW], F32, kind="ExternalOutput")
    afeat = nc.dram_tensor("afeat", [BL, D, HW], F32, kind="ExternalOutput")

    def r128(ap):
        return ap.rearrange("(po pi) f -> pi po f", pi=128)

    with tile.TileContext(nc) as tc:
        with (
            tc.tile_pool(name="consts", bufs=1) as consts,
            tc.tile_pool(name="work", bufs=3) as work,
            tc.tile_pool(name="big", bufs=2) as big,
            tc.tile_pool(name="outp", bufs=2) as outp,
            tc.tile_pool(name="psA", bufs=2, space="PSUM") as psA,
            tc.tile_pool(name="ps1p", bufs=2, space="PSUM") as ps1p,
            tc.tile_pool(name="ps2p", bufs=2, space="PSUM") as ps2p,
        ):
            # ---- constant loads (SP HWDGE queue) ----
            def cload(dram, shape, dt=F32):
                t = consts.tile(shape, dt, tag=dram.name)
                nc.sync.dma_start(t[:], dram.ap())
                return t

            aoht_sb = cload(aoht, [128, 4, BL])
            mre_sb = cload(d_mre, [128, 4, D])
            mim_sb = cload(d_mim, [128, 4, D])
            frp_sb = cload(d_frp, [128, 2, D])
            fip_sb = cload(d_fip, [128, 2, D])
            ut_sb = cload(d_ut, [128, 2, 32])
            vt_sb = cload(d_vt, [128, 2, 32])
            w2b_sb = cload(d_w2b, [32, 128], dt=MMDT)
            c1b_sb = cload(d_c1b, [32, 1])
            c2b_sb = cload(d_c2b, [128, 1])

            # ---- stage A: per-sample G matrices (tiny, fp32 exact) ----
            f1re_sb = consts.tile([128, 2, BL], F32, tag="f1re")
            f1im_sb = consts.tile([128, 2, BL], F32, tag="f1im")
            for dst, mat in ((f1re_sb, mre_sb), (f1im_sb, mim_sb)):
                for m in range(2):
                    ps = psA.tile([128, BL], F32, tag="psA")
                    for po in range(4):
                        nc.tensor.matmul(
                            ps[:], lhsT=mat[:, po, m * 128:(m + 1) * 128],
                            rhs=aoht_sb[:, po, :],
                            start=(po == 0), stop=(po == 3))
                    nc.vector.tensor_copy(dst[:, m, :], ps[:])

            GT_sb = consts.tile([128, 2, BL, 32], MMDT, tag="GT")
            for b in range(BL):
                f1re_b = f1re_sb[:, :, b:b + 1].to_broadcast([128, 2, 32])
                f1im_b = f1im_sb[:, :, b:b + 1].to_broadcast([128, 2, 32])
                L1 = work.tile([128, 2, 32], F32, tag="L1")
                L2 = work.tile([128, 2, 32], F32, tag="L2")
                t1 = work.tile([128, 2, 32], F32, tag="Ltmp")
                nc.vector.tensor_tensor(L1[:], ut_sb[:], f1re_b, Alu.mult)
                nc.vector.tensor_tensor(t1[:], vt_sb[:], f1im_b, Alu.mult)
                nc.vector.tensor_add(L1[:], L1[:], t1[:])
                t2 = work.tile([128, 2, 32], F32, tag="Ltmp")
                nc.vector.tensor_tensor(L2[:], vt_sb[:], f1re_b, Alu.mult)
                nc.vector.tensor_tensor(t2[:], ut_sb[:], f1im_b, Alu.mult)
                nc.vector.tensor_tensor(L2[:], L2[:], t2[:], Alu.subtract)
                for m in range(2):
                    ps = psA.tile([128, 32], F32, tag="psG")
                    nc.tensor.matmul(ps[:], lhsT=frp_sb[:, 0, m * 128:(m + 1) * 128],
                                     rhs=L1[:, 0, :], start=True, stop=False)
                    nc.tensor.matmul(ps[:], lhsT=frp_sb[:, 1, m * 128:(m + 1) * 128],
                                     rhs=L1[:, 1, :], start=False, stop=False)
                    nc.tensor.matmul(ps[:], lhsT=fip_sb[:, 0, m * 128:(m + 1) * 128],
                                     rhs=L2[:, 0, :], start=False, stop=False)
                    nc.tensor.matmul(ps[:], lhsT=fip_sb[:, 1, m * 128:(m + 1) * 128],
                                     rhs=L2[:, 1, :], start=False, stop=True)
                    nc.vector.tensor_copy(GT_sb[:, m, b, :], ps[:])

            # ---- stage B: the heavy per-pixel pipeline ----
            for b in range(BL):
                E = big.tile([128, 2, HW], F32, tag="E")
                nc.scalar.dma_start(E[:], r128(ef.ap()[b]))  # ACT HWDGE queue
                O = outp.tile([128, 2, HW], F32, tag="O")
                for j in range(NT):
                    js = slice(j * TW, (j + 1) * TW)
                    r = work.tile([128, 2, TW], MMDT, tag="r")
                    nc.vector.tensor_scalar(r[:], E[:, :, js], 0.0, None,
                                            Alu.max)
                    ps1 = ps1p.tile([32, TW], F32, tag="ps1")
                    for po in range(2):
                        nc.tensor.matmul(
                            ps1[:],
                            lhsT=GT_sb[:, po, b, :],
                            rhs=r[:, po, :],
                            start=(po == 0), stop=(po == 1))
                    xt = work.tile([32, TW], MMDT, tag="x")
                    nc.scalar.activation(xt[:], ps1[:], Act.Relu,
                                         bias=c1b_sb[:])
                    ps2 = ps2p.tile([128, TW], F32, tag="ps2")
                    nc.tensor.matmul(ps2[:], lhsT=w2b_sb[:],
                                     rhs=xt[:],
                                     start=True, stop=True)
                    am = work.tile([128, TW], F32, tag="am")
                    nc.scalar.activation(am[:], ps2[:], Act.Sigmoid,
                                         bias=c2b_sb[:])
                    nc.vector.tensor_tensor(
                        O[:, 0, js], E[:, 0, js], am[:], Alu.mult)
                    nc.vector.tensor_tensor(
                        O[:, 1, js], E[:, 1, js], am[:], Alu.mult)
                    nc.sync.dma_start(amap.ap()[b][:, js], am[0:1, :])
                nc.gpsimd.dma_start(r128(afeat.ap()[b]), O[:])

    nc.compile()
    return nc


def kernel(entity_feature, attr_one_hot, W_emb, b_emb, conv_w, conv_b,
           conv1_w, conv1_b, conv2_w, conv2_b, h1, h2, s1, s2):
    global LAST_RESULT
    from concourse.bass_utils import run_bass_kernel_spmd

    consts = _host_constants(W_emb, b_emb, conv_w, conv_b, conv1_w, conv1_b,
                             conv2_w, conv2_b, h1, h2, s1, s2)

    ef_full = np.ascontiguousarray(
        np.asarray(entity_feature, np.float32).reshape(B, D, HW))
    aoh_full = np.asarray(attr_one_hot, np.float32)

    in_maps = []
    for c in range(NCORES):
        sl = slice(c * BL, (c + 1) * BL)
        aug = np.zeros((KAUG, BL), np.float32)
        aug[:ATTR] = aoh_full[sl].T
        aug[ATTR] = 1.0
        m = dict(consts)
        m["ef"] = np.ascontiguousarray(ef_full[sl])
        m["aoht"] = np.ascontiguousarray(
            aug.reshape(4, 128, BL).transpose(1, 0, 2))
        in_maps.append(m)

    mm_dtype = os.environ.get("KERNEL_MM_DTYPE", "bf16")
    if mm_dtype == "bf16":
        import ml_dtypes
        for m in in_maps:
            m["w2b"] = m["w2b"].astype(ml_dtypes.bfloat16)
    nc = _build_nc(mm_dtype=mm_dtype)

    trace = bool(int(os.environ.get("KERNEL_TRACE", "0")))
    res = run_bass_kernel_spmd(nc, in_maps, core_ids=list(range(NCORES)),
                               trace=trace)
    LAST_RESULT = res

    amap = np.concatenate([res.results[c]["amap"] for c in range(NCORES)], 0)
    afeat = np.concatenate([res.results[c]["afeat"] for c in range(NCORES)], 0)
    return (np.ascontiguousarray(amap.reshape(B, 1, H, W)),
            np.ascontiguousarray(afeat.reshape(B, D, H, W)))


# revision 6
# speedup vs baseline: 1.1594x; 1.0446x over previous
"""Trainium2 Bass kernel for nn_AttributeBranch.

Math: the CBP (count-sketch -> rFFT -> elementwise mul -> irFFT) block is,
per sample, a LINEAR operator on the channel dim:
    mcb[b] = Circ(sk1[b]) @ P2^T @ feature[b]
where Circ(v)[n, e] = v[(n-e) mod D] (circular convolution) and sk1[b]
depends only on attr_one_hot[b].  Folding conv1_w in gives a per-sample
[32, 256] matrix G_b:
    x[b]       = relu(G_b @ relu(E_b) + c1)          # E_b = entity[b] as [256, HW]
    attr_map   = sigmoid(conv2_w @ x + c2)           # [1, HW]
    attr_feat  = attr_map * E_b
G_b = L1 @ (Fr P2^T) + L2 @ (Fi P2^T), with
    L1 = u * f1re[b] + v * f1im[b],  L2 = v * f1re[b] - u * f1im[b]
    u = conv1_w @ Br, v = conv1_w @ Bi   (full 256-bin real DFT matrices)
and the whole attr_one_hot -> f1re/f1im chain collapses into two
parameter-only [256, 512] matrices (augmented with a bias row).

All data-dependent compute runs on the NeuronCores.  Host only folds
*parameter* matrices and re-lays-out tensors.  Sharding: pure data parallel
over batch (4 samples per core x 8 cores).
"""

import os

import numpy as np

D = 256
H = 56
W = 56
HW = H * W
B = 32
ATTR = 400
NCORES = 8
BL = B // NCORES          # samples per core
NT = 7                    # spatial tiles per sample
TW = HW // NT             # 448 columns per tile
KAUG = 512                # padded (attr_num + bias row) contraction dim

LAST_RESULT = None        # BassKernelResults of the most recent kernel() call


def _swizzle_k(mat):
    """[K, M] with K = n*128 -> [128, n, M] so that dev[pi, po, m] = mat[po*128+pi, m]."""
    k, m = mat.shape
    n = k // 128
    return np.ascontiguousarray(
        mat.reshape(n, 128, m).transpose(1, 0, 2)
    ).astype(np.float32)


def _host_constants(W_emb, b_emb, conv_w, conv_b, conv1_w, conv1_b, conv2_w,
                    conv2_b, h1, h2, s1, s2):
    """Fold all parameter-only matrices (float64 for accuracy)."""
    f8 = np.float64

    def sketch(h, s):
        P = np.zeros((D, D), f8)
        np.add.at(P, (np.arange(D), np.asarray(h)), np.asarray(s, f8))
        return P

    P1m = sketch(h1, s1)
    P2m = sketch(h2, s2)

    k = np.arange(D)
    ang = 2.0 * np.pi * np.outer(k, k) / D
    Fr = np.cos(ang)
    Fi = -np.sin(ang)
    Br = Fr / D
    Bi = Fi / D

    conv_w = np.asarray(conv_w, f8)
    W_emb = np.asarray(W_emb, f8)
    conv1_w = np.asarray(conv1_w, f8)

    u = conv1_w @ Br                      # [32, 256]
    v = conv1_w @ Bi
    FrP = Fr @ P2m.T                      # [256, 256]
    FiP = Fi @ P2m.T

    # attr_one_hot -> f1{re,im} collapsed:  f1 = aoh_aug @ M_aug.T
    pre = P1m.T @ (conv_w @ W_emb)        # [256, 400]
    cvec = P1m.T @ (conv_w @ np.asarray(b_emb, f8) + np.asarray(conv_b, f8))
    Mre = np.zeros((D, KAUG), f8)
    Mim = np.zeros((D, KAUG), f8)
    Mre[:, :ATTR] = Fr @ pre
    Mre[:, ATTR] = Fr @ cvec
    Mim[:, :ATTR] = Fi @ pre
    Mim[:, ATTR] = Fi @ cvec

    return {
        "mre": _swizzle_k(Mre.T),                      # [128, 4, 256]
        "mim": _swizzle_k(Mim.T),
        "frp": _swizzle_k(FrP),                        # [128, 2, 256]
        "fip": _swizzle_k(FiP),
        "ut": _swizzle_k(u.T),                         # [128, 2, 32]
        "vt": _swizzle_k(v.T),
        "w2b": np.tile(
            np.asarray(conv2_w, np.float64).reshape(32, 1), (1, 128)
        ).astype(np.float32),                          # [32, 128]
        "c1b": np.asarray(conv1_b, np.float32).reshape(32, 1),
        "c2b": np.full((128, 1), float(np.asarray(conv2_b).reshape(-1)[0]),
                       np.float32),
    }


def _build_nc(mm_dtype="bf16"):
    import concourse.bacc as bacc
    import concourse.mybir as mybir
    import concourse.tile as tile

    F32 = mybir.dt.float32
    MMDT = {"bf16": mybir.dt.bfloat16,
            "f32r": mybir.dt.float32r,
            "f32": mybir.dt.float32}[mm_dtype]
    Alu = mybir.AluOpType
    Act = mybir.ActivationFunctionType

    nc = bacc.Bacc("TRN2", target_bir_lowering=False, debug=False,
                   num_devices=NCORES)

    ef = nc.dram_tensor("ef", [BL, D, HW], F32, kind="ExternalInput")
    aoht = nc.dram_tensor("aoht", [128, 4, BL], F32, kind="ExternalInput")
    d_mre = nc.dram_tensor("mre", [128, 4, D], F32, kind="ExternalInput")
    d_mim = nc.dram_tensor("mim", [128, 4, D], F32, kind="ExternalInput")
    d_frp = nc.dram_tensor("frp", [128, 2, D], F32, kind="ExternalInput")
    d_fip = nc.dram_tensor("fip", [128, 2, D], F32, kind="ExternalInput")
    d_ut = nc.dram_tensor("ut", [128, 2, 32], F32, kind="ExternalInput")
    d_vt = nc.dram_tensor("vt", [128, 2, 32], F32, kind="ExternalInput")
    d_w2b = nc.dram_tensor("w2b", [32, 128], MMDT, kind="ExternalInput")
    d_c1b = nc.dram_tensor("c1b", [32, 1], F32, kind="ExternalInput")
    d_c2b = nc.dram_tensor("c2b", [128, 1], F32, kind="ExternalInput")
    amap = nc.dram_tensor("amap", [BL, 1, HW], F32, kind="ExternalOutput")
    afeat = nc.dram_tensor("afeat", [BL, D, HW], F32, kind="ExternalOutput")

    def r128(ap):
        return ap.rearrange("(po pi) f -> pi po f", pi=128)

    with tile.TileContext(nc) as tc:
        with (
            tc.tile_pool(name="consts", bufs=1) as consts,
            tc.tile_pool(name="work", bufs=3) as work,
            tc.tile_pool(name="big", bufs=3) as big,
            tc.tile_pool(name="psA", bufs=2, space="PSUM") as psA,
            tc.tile_pool(name="ps1p", bufs=2, space="PSUM") as ps1p,
            tc.tile_pool(name="ps2p", bufs=2, space="PSUM") as ps2p,
        ):
            # ---- constant loads (SP HWDGE queue) ----
            def cload(dram, shape, dt=F32):
                t = consts.tile(shape, dt, tag=dram.name)
                nc.sync.dma_start(t[:], dram.ap())
                return t

            aoht_sb = cload(aoht, [128, 4, BL])
            mre_sb = cload(d_mre, [128, 4, D])
            mim_sb = cload(d_mim, [128, 4, D])
            frp_sb = cload(d_frp, [128, 2, D])
            fip_sb = cload(d_fip, [128, 2, D])
            ut_sb = cload(d_ut, [128, 2, 32])
            vt_sb = cload(d_vt, [128, 2, 32])
            w2b_sb = cload(d_w2b, [32, 128], dt=MMDT)
            c1b_sb = cload(d_c1b, [32, 1])
            c2b_sb = cload(d_c2b, [128, 1])

            # ---- stage A: per-sample G matrices (tiny, fp32 exact) ----
            f1re_sb = consts.tile([128, 2, BL], F32, tag="f1re")
            f1im_sb = consts.tile([128, 2, BL], F32, tag="f1im")
            for dst, mat in ((f1re_sb, mre_sb), (f1im_sb, mim_sb)):
                for m in range(2):
                    ps = psA.tile([128, BL], F32, tag="psA")
                    for po in range(4):
                        nc.tensor.matmul(
                            ps[:], lhsT=mat[:, po, m * 128:(m + 1) * 128],
                            rhs=aoht_sb[:, po, :],
                            start=(po == 0), stop=(po == 3))
                    nc.vector.tensor_copy(dst[:, m, :], ps[:])

            GT_sb = consts.tile([128, 2, BL, 32], MMDT, tag="GT")
            for b in range(BL):
                f1re_b = f1re_sb[:, :, b:b + 1].to_broadcast([128, 2, 32])
                f1im_b = f1im_sb[:, :, b:b + 1].to_broadcast([128, 2, 32])
                L1 = work.tile([128, 2, 32], F32, tag="L1")
                L2 = work.tile([128, 2, 32], F32, tag="L2")
                t1 = work.tile([128, 2, 32], F32, tag="Ltmp")
                nc.vector.tensor_tensor(L1[:], ut_sb[:], f1re_b, Alu.mult)
                nc.vector.tensor_tensor(t1[:], vt_sb[:], f1im_b, Alu.mult)
                nc.vector.tensor_add(L1[:], L1[:], t1[:])
                t2 = work.tile([128, 2, 32], F32, tag="Ltmp")
                nc.vector.tensor_tensor(L2[:], vt_sb[:], f1re_b, Alu.mult)
                nc.vector.tensor_tensor(t2[:], ut_sb[:], f1im_b, Alu.mult)
                nc.vector.tensor_tensor(L2[:], L2[:], t2[:], Alu.subtract)
                for m in range(2):
                    ps = psA.tile([128, 32], F32, tag="psG")
                    nc.tensor.matmul(ps[:], lhsT=frp_sb[:, 0, m * 128:(m + 1) * 128],
                                     rhs=L1[:, 0, :], start=True, stop=False)
                    nc.tensor.matmul(ps[:], lhsT=frp_sb[:, 1, m * 128:(m + 1) * 128],
                                     rhs=L1[:, 1, :], start=False, stop=False)
                    nc.tensor.matmul(ps[:], lhsT=fip_sb[:, 0, m * 128:(m + 1) * 128],
                                     rhs=L2[:, 0, :], start=False, stop=False)
                    nc.tensor.matmul(ps[:], lhsT=fip_sb[:, 1, m * 128:(m + 1) * 128],
                                     rhs=L2[:, 1, :], start=False, stop=True)
                    nc.vector.tensor_copy(GT_sb[:, m, b, :], ps[:])

            # ---- stage B: the heavy per-pixel pipeline ----
            for b in range(BL):
                E = big.tile([128, 2, HW], F32, tag="E")
                eap = r128(ef.ap()[b])
                half = (NT // 2) * TW
                nc.scalar.dma_start(E[:, :, :half], eap[:, :, :half])
                nc.scalar.dma_start(E[:, :, half:], eap[:, :, half:])
                oap = r128(afeat.ap()[b])
                for j in range(NT):
                    js = slice(j * TW, (j + 1) * TW)
                    r = work.tile([128, 2, TW], MMDT, tag="r")
                    nc.vector.tensor_scalar(r[:], E[:, :, js], 0.0, None,
                                            Alu.max)
                    ps1 = ps1p.tile([32, TW], F32, tag="ps1")
                    for po in range(2):
                        nc.tensor.matmul(
                            ps1[:],
                            lhsT=GT_sb[:, po, b, :],
                            rhs=r[:, po, :],
                            start=(po == 0), stop=(po == 1))
                    xt = work.tile([32, TW], MMDT, tag="x")
                    nc.scalar.activation(xt[:], ps1[:], Act.Relu,
                                         bias=c1b_sb[:])
                    ps2 = ps2p.tile([128, TW], F32, tag="ps2")
                    nc.tensor.matmul(ps2[:], lhsT=w2b_sb[:],
                                     rhs=xt[:],
                                     start=True, stop=True)
                    am = work.tile([128, TW], F32, tag="am")
                    nc.scalar.activation(am[:], ps2[:], Act.Sigmoid,
                                         bias=c2b_sb[:])
                    ot = work.tile([128, 2, TW], F32, tag="o")
                    nc.vector.tensor_tensor(
                        ot[:, 0, :], E[:, 0, js], am[:], Alu.mult)
                    nc.vector.tensor_tensor(
                        ot[:, 1, :], E[:, 1, js], am[:], Alu.mult)
                    nc.sync.dma_start(amap.ap()[b][:, js], am[0:1, :])
                    nc.gpsimd.dma_start(oap[:, :, js], ot[:])

    nc.compile()
    return nc


def kernel(entity_feature, attr_one_hot, W_emb, b_emb, conv_w, conv_b,
           conv1_w, conv1_b, conv2_w, conv2_b, h1, h2, s1, s2):
    global LAST_RESULT
    from concourse.bass_utils import run_bass_kernel_spmd

    consts = _host_constants(W_emb, b_emb, conv_w, conv_b, conv1_w, conv1_b,
                             conv2_w, conv2_b, h1, h2, s1, s2)

    ef_full = np.ascontiguousarray(
        np.asarray(entity_feature, np.float32).reshape(B, D, HW))
    aoh_full = np.asarray(attr_one_hot, np.float32)

    in_maps = []
    for c in range(NCORES):
        sl = slice(c * BL, (c + 1) * BL)
        aug = np.zeros((KAUG, BL), np.float32)
        aug[:ATTR] = aoh_full[sl].T
        aug[ATTR] = 1.0
        m = dict(consts)
        m["ef"] = np.ascontiguousarray(ef_full[sl])
        m["aoht"] = np.ascontiguousarray(
            aug.reshape(4, 128, BL).transpose(1, 0, 2))
        in_maps.append(m)

    mm_dtype = os.environ.get("KERNEL_MM_DTYPE", "bf16")
    if mm_dtype == "bf16":
        import ml_dtypes
        for m in in_maps:
            m["w2b"] = m["w2b"].astype(ml_dtypes.bfloat16)
    nc = _build_nc(mm_dtype=mm_dtype)

    trace = bool(int(os.environ.get("KERNEL_TRACE", "0")))
    res = run_bass_kernel_spmd(nc, in_maps, core_ids=list(range(NCORES)),
                               trace=trace)
    LAST_RESULT = res

    amap = np.concatenate([res.results[c]["amap"] for c in range(NCORES)], 0)
    afeat = np.concatenate([res.results[c]["afeat"] for c in range(NCORES)], 0)
    return (np.ascontiguousarray(amap.reshape(B, 1, H, W)),
            np.ascontiguousarray(afeat.reshape(B, D, H, W)))


# revision 7
# speedup vs baseline: 1.3081x; 1.1282x over previous
"""Trainium2 Bass kernel for nn_AttributeBranch.

Math: the CBP (count-sketch -> rFFT -> elementwise mul -> irFFT) block is,
per sample, a LINEAR operator on the channel dim:
    mcb[b] = Circ(sk1[b]) @ P2^T @ feature[b]
where Circ(v)[n, e] = v[(n-e) mod D] (circular convolution) and sk1[b]
depends only on attr_one_hot[b].  Folding conv1_w in gives a per-sample
[32, 256] matrix G_b:
    x[b]       = relu(G_b @ relu(E_b) + c1)          # E_b = entity[b] as [256, HW]
    attr_map   = sigmoid(conv2_w @ x + c2)           # [1, HW]
    attr_feat  = attr_map * E_b
G_b = L1 @ (Fr P2^T) + L2 @ (Fi P2^T), with
    L1 = u * f1re[b] + v * f1im[b],  L2 = v * f1re[b] - u * f1im[b]
    u = conv1_w @ Br, v = conv1_w @ Bi   (full 256-bin real DFT matrices)
and the whole attr_one_hot -> f1re/f1im chain collapses into two
parameter-only [256, 512] matrices (augmented with a bias row).

All data-dependent compute runs on the NeuronCores.  Host only folds
*parameter* matrices and re-lays-out tensors.  Sharding: pure data parallel
over batch (4 samples per core x 8 cores).
"""

import os

import numpy as np

D = 256
H = 56
W = 56
HW = H * W
B = 32
ATTR = 400
NCORES = 8
BL = B // NCORES          # samples per core
NT = 7                    # spatial tiles per sample
TW = HW // NT             # 448 columns per tile
KAUG = 512                # padded (attr_num + bias row) contraction dim

LAST_RESULT = None        # BassKernelResults of the most recent kernel() call


def _swizzle_k(mat):
    """[K, M] with K = n*128 -> [128, n, M] so that dev[pi, po, m] = mat[po*128+pi, m]."""
    k, m = mat.shape
    n = k // 128
    return np.ascontiguousarray(
        mat.reshape(n, 128, m).transpose(1, 0, 2)
    ).astype(np.float32)


def _host_constants(W_emb, b_emb, conv_w, conv_b, conv1_w, conv1_b, conv2_w,
                    conv2_b, h1, h2, s1, s2):
    """Fold all parameter-only matrices (float64 for accuracy)."""
    f8 = np.float64

    def sketch(h, s):
        P = np.zeros((D, D), f8)
        np.add.at(P, (np.arange(D), np.asarray(h)), np.asarray(s, f8))
        return P

    P1m = sketch(h1, s1)
    P2m = sketch(h2, s2)

    k = np.arange(D)
    ang = 2.0 * np.pi * np.outer(k, k) / D
    Fr = np.cos(ang)
    Fi = -np.sin(ang)
    Br = Fr / D
    Bi = Fi / D

    conv_w = np.asarray(conv_w, f8)
    W_emb = np.asarray(W_emb, f8)
    conv1_w = np.asarray(conv1_w, f8)

    u = conv1_w @ Br                      # [32, 256]
    v = conv1_w @ Bi
    FrP = Fr @ P2m.T                      # [256, 256]
    FiP = Fi @ P2m.T

    # attr_one_hot -> f1{re,im} collapsed:  f1 = aoh_aug @ M_aug.T
    pre = P1m.T @ (conv_w @ W_emb)        # [256, 400]
    cvec = P1m.T @ (conv_w @ np.asarray(b_emb, f8) + np.asarray(conv_b, f8))
    Mre = np.zeros((D, KAUG), f8)
    Mim = np.zeros((D, KAUG), f8)
    Mre[:, :ATTR] = Fr @ pre
    Mre[:, ATTR] = Fr @ cvec
    Mim[:, :ATTR] = Fi @ pre
    Mim[:, ATTR] = Fi @ cvec

    return {
        "mre": _swizzle_k(Mre.T),                      # [128, 4, 256]
        "mim": _swizzle_k(Mim.T),
        "frp": _swizzle_k(FrP),                        # [128, 2, 256]
        "fip": _swizzle_k(FiP),
        "ut": _swizzle_k(u.T),                         # [128, 2, 32]
        "vt": _swizzle_k(v.T),
        "w2b": np.tile(
            np.asarray(conv2_w, np.float64).reshape(32, 1), (1, 128)
        ).astype(np.float32),                          # [32, 128]
        "c1b": np.asarray(conv1_b, np.float32).reshape(32, 1),
        "c2b": np.full((128, 1), float(np.asarray(conv2_b).reshape(-1)[0]),
                       np.float32),
    }


OFF_MRE, OFF_MIM = 0, 1024
OFF_FRP, OFF_FIP = 2048, 2560
OFF_UT, OFF_VT = 3072, 3136
OFF_W2B, OFF_C1B, OFF_C2B, OFF_AOHT = 3200, 3328, 3329, 3330
CW = OFF_AOHT + 4 * BL


def _build_nc(mm_dtype="bf16", e_bf16=True):
    import concourse.bacc as bacc
    import concourse.mybir as mybir
    import concourse.tile as tile

    F32 = mybir.dt.float32
    BF16 = mybir.dt.bfloat16
    MMDT = {"bf16": BF16,
            "f32r": mybir.dt.float32r,
            "f32": mybir.dt.float32}[mm_dtype]
    EDT = BF16 if e_bf16 else F32
    Alu = mybir.AluOpType
    Act = mybir.ActivationFunctionType

    nc = bacc.Bacc("TRN2", target_bir_lowering=False, debug=False,
                   num_devices=NCORES)

    ef = nc.dram_tensor("ef", [BL, D, HW], F32, kind="ExternalInput")
    cblob = nc.dram_tensor("cblob", [128, CW], F32, kind="ExternalInput")
    amap = nc.dram_tensor("amap", [BL, 1, HW], F32, kind="ExternalOutput")
    afeat = nc.dram_tensor("afeat", [BL, D, HW], F32, kind="ExternalOutput")

    def r128(ap):
        return ap.rearrange("(po pi) f -> pi po f", pi=128)

    with tile.TileContext(nc) as tc:
        with (
            tc.tile_pool(name="consts", bufs=1) as consts,
            tc.tile_pool(name="work", bufs=3) as work,
            tc.tile_pool(name="opool", bufs=8) as opool,
            tc.tile_pool(name="big", bufs=BL) as big,
            tc.tile_pool(name="psA", bufs=2, space="PSUM") as psA,
            tc.tile_pool(name="ps1p", bufs=2, space="PSUM") as ps1p,
            tc.tile_pool(name="ps2p", bufs=2, space="PSUM") as ps2p,
        ):
            # ---- one packed constant load (SP HWDGE ring) ----
            blob = consts.tile([128, CW], F32, tag="cblob")
            nc.sync.dma_start(blob[:], cblob.ap())
            mre_sb = blob[:, OFF_MRE:OFF_MRE + 1024].rearrange(
                "pi (po m) -> pi po m", po=4)
            mim_sb = blob[:, OFF_MIM:OFF_MIM + 1024].rearrange(
                "pi (po m) -> pi po m", po=4)
            frp_sb = blob[:, OFF_FRP:OFF_FRP + 512].rearrange(
                "pi (po m) -> pi po m", po=2)
            fip_sb = blob[:, OFF_FIP:OFF_FIP + 512].rearrange(
                "pi (po m) -> pi po m", po=2)
            ut_sb = blob[:, OFF_UT:OFF_UT + 64].rearrange(
                "pi (po m) -> pi po m", po=2)
            vt_sb = blob[:, OFF_VT:OFF_VT + 64].rearrange(
                "pi (po m) -> pi po m", po=2)
            c1b_sb = blob[:32, OFF_C1B:OFF_C1B + 1]
            c2b_sb = blob[:, OFF_C2B:OFF_C2B + 1]
            aoht_sb = blob[:, OFF_AOHT:OFF_AOHT + 4 * BL].rearrange(
                "pi (po b) -> pi po b", po=4)
            w2b_sb = consts.tile([32, 128], MMDT, tag="w2b")
            nc.vector.tensor_copy(w2b_sb[:], blob[:32, OFF_W2B:OFF_W2B + 128])

            # ---- prefetch all per-sample inputs (SWDGE ring, cast to EDT) ----
            E_tiles = []
            for b in range(BL):
                E = big.tile([128, 2, HW], EDT, tag="E")
                if e_bf16:
                    nc.gpsimd.dma_start(E[:], r128(ef.ap()[b]))
                else:
                    nc.scalar.dma_start(E[:], r128(ef.ap()[b]))
                E_tiles.append(E)

            # ---- stage A: per-sample G matrices (tiny, fp32 exact) ----
            f1re_sb = consts.tile([128, 2, BL], F32, tag="f1re")
            f1im_sb = consts.tile([128, 2, BL], F32, tag="f1im")
            for dst, mat in ((f1re_sb, mre_sb), (f1im_sb, mim_sb)):
                for m in range(2):
                    ps = psA.tile([128, BL], F32, tag="psA")
                    for po in range(4):
                        nc.tensor.matmul(
                            ps[:], lhsT=mat[:, po, m * 128:(m + 1) * 128],
                            rhs=aoht_sb[:, po, :],
                            start=(po == 0), stop=(po == 3))
                    nc.vector.tensor_copy(dst[:, m, :], ps[:])

            GT_sb = consts.tile([128, 2, BL, 32], MMDT, tag="GT")
            for b in range(BL):
                f1re_b = f1re_sb[:, :, b:b + 1].to_broadcast([128, 2, 32])
                f1im_b = f1im_sb[:, :, b:b + 1].to_broadcast([128, 2, 32])
                L1 = work.tile([128, 2, 32], F32, tag="L1")
                L2 = work.tile([128, 2, 32], F32, tag="L2")
                t1 = work.tile([128, 2, 32], F32, tag="Ltmp")
                nc.vector.tensor_tensor(L1[:], ut_sb[:], f1re_b, Alu.mult)
                nc.vector.tensor_tensor(t1[:], vt_sb[:], f1im_b, Alu.mult)
                nc.vector.tensor_add(L1[:], L1[:], t1[:])
                t2 = work.tile([128, 2, 32], F32, tag="Ltmp")
                nc.vector.tensor_tensor(L2[:], vt_sb[:], f1re_b, Alu.mult)
                nc.vector.tensor_tensor(t2[:], ut_sb[:], f1im_b, Alu.mult)
                nc.vector.tensor_tensor(L2[:], L2[:], t2[:], Alu.subtract)
                for m in range(2):
                    ps = psA.tile([128, 32], F32, tag="psG")
                    nc.tensor.matmul(ps[:], lhsT=frp_sb[:, 0, m * 128:(m + 1) * 128],
                                     rhs=L1[:, 0, :], start=True, stop=False)
                    nc.tensor.matmul(ps[:], lhsT=frp_sb[:, 1, m * 128:(m + 1) * 128],
                                     rhs=L1[:, 1, :], start=False, stop=False)
                    nc.tensor.matmul(ps[:], lhsT=fip_sb[:, 0, m * 128:(m + 1) * 128],
                                     rhs=L2[:, 0, :], start=False, stop=False)
                    nc.tensor.matmul(ps[:], lhsT=fip_sb[:, 1, m * 128:(m + 1) * 128],
                                     rhs=L2[:, 1, :], start=False, stop=True)
                    nc.vector.tensor_copy(GT_sb[:, m, b, :], ps[:])

            # ---- stage B: the heavy per-pixel pipeline ----
            for b in range(BL):
                E = E_tiles[b]
                oap = r128(afeat.ap()[b])
                for j in range(NT):
                    js = slice(j * TW, (j + 1) * TW)
                    r = work.tile([128, 2, TW], MMDT, tag="r")
                    nc.vector.tensor_scalar(r[:], E[:, :, js], 0.0, None,
                                            Alu.max)
                    ps1 = ps1p.tile([32, TW], F32, tag="ps1")
                    for po in range(2):
                        nc.tensor.matmul(
                            ps1[:],
                            lhsT=GT_sb[:, po, b, :],
                            rhs=r[:, po, :],
                            start=(po == 0), stop=(po == 1))
                    xt = work.tile([32, TW], MMDT, tag="x")
                    nc.scalar.activation(xt[:], ps1[:], Act.Relu,
                                         bias=c1b_sb[:])
                    ps2 = ps2p.tile([128, TW], F32, tag="ps2")
                    nc.tensor.matmul(ps2[:], lhsT=w2b_sb[:],
                                     rhs=xt[:],
                                     start=True, stop=True)
                    am = work.tile([128, TW], F32, tag="am")
                    nc.scalar.activation(am[:], ps2[:], Act.Sigmoid,
                                         bias=c2b_sb[:])
                    ot = opool.tile([128, 2, TW], F32, tag="o")
                    nc.vector.tensor_tensor(
                        ot[:, 0, :], E[:, 0, js], am[:], Alu.mult)
                    nc.vector.tensor_tensor(
                        ot[:, 1, :], E[:, 1, js], am[:], Alu.mult)
                    nc.sync.dma_start(amap.ap()[b][:, js], am[0:1, :])
                    nc.sync.dma_start(oap[:, :, js], ot[:])

    nc.compile()
    return nc


def kernel(entity_feature, attr_one_hot, W_emb, b_emb, conv_w, conv_b,
           conv1_w, conv1_b, conv2_w, conv2_b, h1, h2, s1, s2):
    global LAST_RESULT
    from concourse.bass_utils import run_bass_kernel_spmd

    consts = _host_constants(W_emb, b_emb, conv_w, conv_b, conv1_w, conv1_b,
                             conv2_w, conv2_b, h1, h2, s1, s2)

    ef_full = np.ascontiguousarray(
        np.asarray(entity_feature, np.float32).reshape(B, D, HW))
    aoh_full = np.asarray(attr_one_hot, np.float32)

    blob0 = np.zeros((128, CW), np.float32)
    blob0[:, OFF_MRE:OFF_MRE + 1024] = consts["mre"].reshape(128, 1024)
    blob0[:, OFF_MIM:OFF_MIM + 1024] = consts["mim"].reshape(128, 1024)
    blob0[:, OFF_FRP:OFF_FRP + 512] = consts["frp"].reshape(128, 512)
    blob0[:, OFF_FIP:OFF_FIP + 512] = consts["fip"].reshape(128, 512)
    blob0[:, OFF_UT:OFF_UT + 64] = consts["ut"].reshape(128, 64)
    blob0[:, OFF_VT:OFF_VT + 64] = consts["vt"].reshape(128, 64)
    blob0[:32, OFF_W2B:OFF_W2B + 128] = consts["w2b"]
    blob0[:32, OFF_C1B] = consts["c1b"][:, 0]
    blob0[:, OFF_C2B] = consts["c2b"][:, 0]

    in_maps = []
    for c in range(NCORES):
        sl = slice(c * BL, (c + 1) * BL)
        aug = np.zeros((KAUG, BL), np.float32)
        aug[:ATTR] = aoh_full[sl].T
        aug[ATTR] = 1.0
        blob = blob0.copy()
        blob[:, OFF_AOHT:OFF_AOHT + 4 * BL] = (
            aug.reshape(4, 128, BL).transpose(1, 0, 2).reshape(128, 4 * BL))
        in_maps.append({"ef": np.ascontiguousarray(ef_full[sl]),
                        "cblob": blob})

    mm_dtype = os.environ.get("KERNEL_MM_DTYPE", "bf16")
    e_bf16 = os.environ.get("KERNEL_E_BF16", "1") == "1"
    nc = _build_nc(mm_dtype=mm_dtype, e_bf16=e_bf16)

    trace = bool(int(os.environ.get("KERNEL_TRACE", "0")))
    res = run_bass_kernel_spmd(nc, in_maps, core_ids=list(range(NCORES)),
                               trace=trace)
    LAST_RESULT = res

    amap = np.concatenate([res.results[c]["amap"] for c in range(NCORES)], 0)
    afeat = np.concatenate([res.results[c]["afeat"] for c in range(NCORES)], 0)
    return (np.ascontiguousarray(amap.reshape(B, 1, H, W)),
            np.ascontiguousarray(afeat.reshape(B, D, H, W)))


# revision 9
# speedup vs baseline: 1.4435x; 1.1036x over previous
"""Trainium2 Bass kernel for nn_AttributeBranch.

Math: the CBP (count-sketch -> rFFT -> elementwise mul -> irFFT) block is,
per sample, a LINEAR operator on the channel dim:
    mcb[b] = Circ(sk1[b]) @ P2^T @ feature[b]
where Circ(v)[n, e] = v[(n-e) mod D] (circular convolution) and sk1[b]
depends only on attr_one_hot[b].  Folding conv1_w in gives a per-sample
[32, 256] matrix G_b:
    x[b]       = relu(G_b @ relu(E_b) + c1)          # E_b = entity[b] as [256, HW]
    attr_map   = sigmoid(conv2_w @ x + c2)           # [1, HW]
    attr_feat  = attr_map * E_b
G_b = L1 @ (Fr P2^T) + L2 @ (Fi P2^T), with
    L1 = u * f1re[b] + v * f1im[b],  L2 = v * f1re[b] - u * f1im[b]
    u = conv1_w @ Br, v = conv1_w @ Bi   (full 256-bin real DFT matrices)
and the whole attr_one_hot -> f1re/f1im chain collapses into two
parameter-only [256, 512] matrices (augmented with a bias row).

All data-dependent compute runs on the NeuronCores.  Host only folds
*parameter* matrices and re-lays-out tensors.  Sharding: pure data parallel
over batch (4 samples per core x 8 cores).
"""

import os

import numpy as np

D = 256
H = 56
W = 56
HW = H * W
B = 32
ATTR = 400
NCORES = 8
BL = B // NCORES          # samples per core
NT = 7                    # spatial tiles per sample
TW = HW // NT             # 448 columns per tile
KAUG = 512                # padded (attr_num + bias row) contraction dim

LAST_RESULT = None        # BassKernelResults of the most recent kernel() call


def _swizzle_k(mat):
    """[K, M] with K = n*128 -> [128, n, M] so that dev[pi, po, m] = mat[po*128+pi, m]."""
    k, m = mat.shape
    n = k // 128
    return np.ascontiguousarray(
        mat.reshape(n, 128, m).transpose(1, 0, 2)
    ).astype(np.float32)


def _host_constants(W_emb, b_emb, conv_w, conv_b, conv1_w, conv1_b, conv2_w,
                    conv2_b, h1, h2, s1, s2):
    """Fold all parameter-only matrices (float64 for accuracy)."""
    f8 = np.float64

    def sketch(h, s):
        P = np.zeros((D, D), f8)
        np.add.at(P, (np.arange(D), np.asarray(h)), np.asarray(s, f8))
        return P

    P1m = sketch(h1, s1)
    P2m = sketch(h2, s2)

    k = np.arange(D)
    ang = 2.0 * np.pi * np.outer(k, k) / D
    Fr = np.cos(ang)
    Fi = -np.sin(ang)
    Br = Fr / D
    Bi = Fi / D

    conv_w = np.asarray(conv_w, f8)
    W_emb = np.asarray(W_emb, f8)
    conv1_w = np.asarray(conv1_w, f8)

    u = conv1_w @ Br                      # [32, 256]
    v = conv1_w @ Bi
    FrP = Fr @ P2m.T                      # [256, 256]
    FiP = Fi @ P2m.T

    # attr_one_hot -> f1{re,im} collapsed:  f1 = aoh_aug @ M_aug.T
    pre = P1m.T @ (conv_w @ W_emb)        # [256, 400]
    cvec = P1m.T @ (conv_w @ np.asarray(b_emb, f8) + np.asarray(conv_b, f8))
    Mre = np.zeros((D, KAUG), f8)
    Mim = np.zeros((D, KAUG), f8)
    Mre[:, :ATTR] = Fr @ pre
    Mre[:, ATTR] = Fr @ cvec
    Mim[:, :ATTR] = Fi @ pre
    Mim[:, ATTR] = Fi @ cvec

    return {
        "mre": _swizzle_k(Mre.T),                      # [128, 4, 256]
        "mim": _swizzle_k(Mim.T),
        "frp": _swizzle_k(FrP),                        # [128, 2, 256]
        "fip": _swizzle_k(FiP),
        "ut": _swizzle_k(u.T),                         # [128, 2, 32]
        "vt": _swizzle_k(v.T),
        "w2b": np.tile(
            np.asarray(conv2_w, np.float64).reshape(32, 1), (1, 128)
        ).astype(np.float32),                          # [32, 128]
        "c1b": np.asarray(conv1_b, np.float32).reshape(32, 1),
        "c2b": np.full((128, 1), float(np.asarray(conv2_b).reshape(-1)[0]),
                       np.float32),
    }


OFF_MRE, OFF_MIM = 0, 1024
OFF_FRP, OFF_FIP = 2048, 2560
OFF_UT, OFF_VT = 3072, 3136
OFF_W2B, OFF_C1B, OFF_C2B, OFF_AOHT = 3200, 3328, 3329, 3330
CW = OFF_AOHT + 4 * BL


def _build_nc(mm_dtype="bf16", e_bf16=True):
    import concourse.bacc as bacc
    import concourse.mybir as mybir
    import concourse.tile as tile

    F32 = mybir.dt.float32
    BF16 = mybir.dt.bfloat16
    MMDT = {"bf16": BF16,
            "f32r": mybir.dt.float32r,
            "f32": mybir.dt.float32}[mm_dtype]
    EDT = BF16 if e_bf16 else F32
    Alu = mybir.AluOpType
    Act = mybir.ActivationFunctionType

    nc = bacc.Bacc("TRN2", target_bir_lowering=False, debug=False,
                   num_devices=NCORES)

    ef = nc.dram_tensor("ef", [BL, D, HW], F32, kind="ExternalInput")
    cblob = nc.dram_tensor("cblob", [128, CW], F32, kind="ExternalInput")
    amap = nc.dram_tensor("amap", [BL, 1, HW], F32, kind="ExternalOutput")
    afeat = nc.dram_tensor("afeat", [BL, D, HW], F32, kind="ExternalOutput")

    def r128(ap):
        return ap.rearrange("(po pi) f -> pi po f", pi=128)

    with tile.TileContext(nc) as tc:
        with (
            tc.tile_pool(name="consts", bufs=1) as consts,
            tc.tile_pool(name="work", bufs=3) as work,
            tc.tile_pool(name="opool", bufs=8) as opool,
            tc.tile_pool(name="big", bufs=BL) as big,
            tc.tile_pool(name="psA", bufs=2, space="PSUM") as psA,
            tc.tile_pool(name="ps1p", bufs=2, space="PSUM") as ps1p,
            tc.tile_pool(name="ps2p", bufs=2, space="PSUM") as ps2p,
        ):
            # ---- one packed constant load (SP HWDGE ring) ----
            blob = consts.tile([128, CW], F32, tag="cblob")
            nc.sync.dma_start(blob[:], cblob.ap())
            mre_sb = blob[:, OFF_MRE:OFF_MRE + 1024].rearrange(
                "pi (po m) -> pi po m", po=4)
            mim_sb = blob[:, OFF_MIM:OFF_MIM + 1024].rearrange(
                "pi (po m) -> pi po m", po=4)
            frp_sb = blob[:, OFF_FRP:OFF_FRP + 512].rearrange(
                "pi (po m) -> pi po m", po=2)
            fip_sb = blob[:, OFF_FIP:OFF_FIP + 512].rearrange(
                "pi (po m) -> pi po m", po=2)
            ut_sb = blob[:, OFF_UT:OFF_UT + 64].rearrange(
                "pi (po m) -> pi po m", po=2)
            vt_sb = blob[:, OFF_VT:OFF_VT + 64].rearrange(
                "pi (po m) -> pi po m", po=2)
            c1b_sb = blob[:32, OFF_C1B:OFF_C1B + 1]
            c2b_sb = blob[:, OFF_C2B:OFF_C2B + 1]
            aoht_sb = blob[:, OFF_AOHT:OFF_AOHT + 4 * BL].rearrange(
                "pi (po b) -> pi po b", po=4)
            w2b_sb = consts.tile([128, 128], MMDT, tag="w2b")
            nc.vector.memset(w2b_sb[:], 0.0)
            nc.vector.tensor_copy(w2b_sb[:32, :], blob[:32, OFF_W2B:OFF_W2B + 128])
            xts = []
            for i in range(3):
                xt = consts.tile([128, TW], MMDT, tag=f"xt{i}")
                nc.vector.memset(xt[:], 0.0)
                xts.append(xt)

            # ---- prefetch all per-sample inputs (SWDGE ring, cast to EDT) ----
            E_tiles = []
            for b in range(BL):
                E = big.tile([128, 2, HW], EDT, tag="E")
                eng = nc.gpsimd if e_bf16 else nc.scalar
                eap = r128(ef.ap()[b])
                if b == 0:
                    half = (NT // 2) * TW
                    eng.dma_start(E[:, :, :half], eap[:, :, :half])
                    eng.dma_start(E[:, :, half:], eap[:, :, half:])
                else:
                    eng.dma_start(E[:], eap)
                E_tiles.append(E)

            # ---- stage A: per-sample G matrices (tiny, fp32 exact) ----
            f1re_sb = consts.tile([128, 2, BL], F32, tag="f1re")
            f1im_sb = consts.tile([128, 2, BL], F32, tag="f1im")
            for dst, mat in ((f1re_sb, mre_sb), (f1im_sb, mim_sb)):
                for m in range(2):
                    ps = psA.tile([128, BL], F32, tag="psA")
                    for po in range(4):
                        nc.tensor.matmul(
                            ps[:], lhsT=mat[:, po, m * 128:(m + 1) * 128],
                            rhs=aoht_sb[:, po, :],
                            start=(po == 0), stop=(po == 3))
                    nc.vector.tensor_copy(dst[:, m, :], ps[:])

            GT_sb = consts.tile([128, 2, BL, 32], MMDT, tag="GT")
            for b in range(BL):
                f1re_b = f1re_sb[:, :, b:b + 1].to_broadcast([128, 2, 32])
                f1im_b = f1im_sb[:, :, b:b + 1].to_broadcast([128, 2, 32])
                L1 = work.tile([128, 2, 32], F32, tag="L1")
                L2 = work.tile([128, 2, 32], F32, tag="L2")
                t1 = work.tile([128, 2, 32], F32, tag="Ltmp")
                nc.vector.tensor_tensor(L1[:], ut_sb[:], f1re_b, Alu.mult)
                nc.vector.tensor_tensor(t1[:], vt_sb[:], f1im_b, Alu.mult)
                nc.vector.tensor_add(L1[:], L1[:], t1[:])
                t2 = work.tile([128, 2, 32], F32, tag="Ltmp")
                nc.vector.tensor_tensor(L2[:], vt_sb[:], f1re_b, Alu.mult)
                nc.vector.tensor_tensor(t2[:], ut_sb[:], f1im_b, Alu.mult)
                nc.vector.tensor_tensor(L2[:], L2[:], t2[:], Alu.subtract)
                for m in range(2):
                    ps = psA.tile([128, 32], F32, tag="psG")
                    nc.tensor.matmul(ps[:], lhsT=frp_sb[:, 0, m * 128:(m + 1) * 128],
                                     rhs=L1[:, 0, :], start=True, stop=False)
                    nc.tensor.matmul(ps[:], lhsT=frp_sb[:, 1, m * 128:(m + 1) * 128],
                                     rhs=L1[:, 1, :], start=False, stop=False)
                    nc.tensor.matmul(ps[:], lhsT=fip_sb[:, 0, m * 128:(m + 1) * 128],
                                     rhs=L2[:, 0, :], start=False, stop=False)
                    nc.tensor.matmul(ps[:], lhsT=fip_sb[:, 1, m * 128:(m + 1) * 128],
                                     rhs=L2[:, 1, :], start=False, stop=True)
                    nc.vector.tensor_copy(GT_sb[:, m, b, :], ps[:])

            # ---- stage B: the heavy per-pixel pipeline ----
            for b in range(BL):
                E = E_tiles[b]
                oap = r128(afeat.ap()[b])
                for j in range(NT):
                    js = slice(j * TW, (j + 1) * TW)
                    r = work.tile([128, 2, TW], MMDT, tag="r")
                    nc.vector.tensor_scalar(r[:], E[:, :, js], 0.0, None,
                                            Alu.max)
                    ps1 = ps1p.tile([32, TW], F32, tag="ps1")
                    for po in range(2):
                        nc.tensor.matmul(
                            ps1[:],
                            lhsT=GT_sb[:, po, b, :],
                            rhs=r[:, po, :],
                            start=(po == 0), stop=(po == 1))
                    xt = xts[(b * NT + j) % 3]
                    nc.scalar.activation(xt[:32, :], ps1[:], Act.Relu,
                                         bias=c1b_sb[:])
                    ps2 = ps2p.tile([128, TW], F32, tag="ps2")
                    nc.tensor.matmul(ps2[:], lhsT=w2b_sb[:],
                                     rhs=xt[:],
                                     start=True, stop=True)
                    am = work.tile([128, TW], F32, tag="am")
                    nc.scalar.activation(am[:], ps2[:], Act.Sigmoid,
                                         bias=c2b_sb[:])
                    ot = opool.tile([128, 2, TW], F32, tag="o")
                    nc.vector.tensor_tensor(
                        ot[:], E[:, :, js],
                        am[:, None, :].to_broadcast([128, 2, TW]), Alu.mult)
                    nc.sync.dma_start(amap.ap()[b][:, js], am[0:1, :])
                    nc.sync.dma_start(oap[:, :, js], ot[:])

    nc.compile()
    return nc


def kernel(entity_feature, attr_one_hot, W_emb, b_emb, conv_w, conv_b,
           conv1_w, conv1_b, conv2_w, conv2_b, h1, h2, s1, s2):
    global LAST_RESULT
    from concourse.bass_utils import run_bass_kernel_spmd

    consts = _host_constants(W_emb, b_emb, conv_w, conv_b, conv1_w, conv1_b,
                             conv2_w, conv2_b, h1, h2, s1, s2)

    ef_full = np.ascontiguousarray(
        np.asarray(entity_feature, np.float32).reshape(B, D, HW))
    aoh_full = np.asarray(attr_one_hot, np.float32)

    blob0 = np.zeros((128, CW), np.float32)
    blob0[:, OFF_MRE:OFF_MRE + 1024] = consts["mre"].reshape(128, 1024)
    blob0[:, OFF_MIM:OFF_MIM + 1024] = consts["mim"].reshape(128, 1024)
    blob0[:, OFF_FRP:OFF_FRP + 512] = consts["frp"].reshape(128, 512)
    blob0[:, OFF_FIP:OFF_FIP + 512] = consts["fip"].reshape(128, 512)
    blob0[:, OFF_UT:OFF_UT + 64] = consts["ut"].reshape(128, 64)
    blob0[:, OFF_VT:OFF_VT + 64] = consts["vt"].reshape(128, 64)
    blob0[:32, OFF_W2B:OFF_W2B + 128] = consts["w2b"]
    blob0[:32, OFF_C1B] = consts["c1b"][:, 0]
    blob0[:, OFF_C2B] = consts["c2b"][:, 0]

    in_maps = []
    for c in range(NCORES):
        sl = slice(c * BL, (c + 1) * BL)
        aug = np.zeros((KAUG, BL), np.float32)
        aug[:ATTR] = aoh_full[sl].T
        aug[ATTR] = 1.0
        blob = blob0.copy()
        blob[:, OFF_AOHT:OFF_AOHT + 4 * BL] = (
            aug.reshape(4, 128, BL).transpose(1, 0, 2).reshape(128, 4 * BL))
        in_maps.append({"ef": np.ascontiguousarray(ef_full[sl]),
                        "cblob": blob})

    mm_dtype = os.environ.get("KERNEL_MM_DTYPE", "bf16")
    e_bf16 = os.environ.get("KERNEL_E_BF16", "1") == "1"
    nc = _build_nc(mm_dtype=mm_dtype, e_bf16=e_bf16)

    trace = bool(int(os.environ.get("KERNEL_TRACE", "0")))
    res = run_bass_kernel_spmd(nc, in_maps, core_ids=list(range(NCORES)),
                               trace=trace)
    LAST_RESULT = res

    amap = np.concatenate([res.results[c]["amap"] for c in range(NCORES)], 0)
    afeat = np.concatenate([res.results[c]["afeat"] for c in range(NCORES)], 0)
    return (np.ascontiguousarray(amap.reshape(B, 1, H, W)),
            np.ascontiguousarray(afeat.reshape(B, D, H, W)))
